# revision 1
# baseline (speedup 1.0000x reference)
"""BatchHardTripletLoss on 8 trn2 NeuronCores (Bass/Tile, SPMD data-parallel).

Strategy: shard anchor rows across cores (512 rows/core). Every core gets the
full transposed embeddings (the "all-gather" is free since the host distributes
full inputs). The pos/neg label masking is folded INTO the Gram matmul via
scaled one-hot label encodings:

    psum[i, j] = e_i . e_j  -  4 * [l_i == l_j]        (e row-normalized)

so for each anchor row i:
    reduce_min(psum[i, :]) = (min sim over positives) - 4   -> hardest positive
    reduce_max(psum[i, :]) =  max sim over negatives        -> hardest negative
(the -4 shift pushes the positive entries strictly below every negative entry:
sims live in [-1, 1]).  per-anchor loss = relu(max - min - 4 + margin) * valid.
Validity (anchor has >=1 other positive and >=1 negative) depends only on
labels and is computed host-side, shipped as a 0/1 mask.

Cross-core reduction: each core returns NM partial sums (one per 128-row
tile); the host adds the 8*NM floats and divides by n_valid.

Implementation notes (trn2 codegen constraints):
  - engine instructions have tiny sync-event budgets (matmul: 1 wait,
    DVE copy/reduce: 1 wait, ACT: 2 waits).  Cross-engine dependency fan-in
    is funneled through tiny "absorber" ops so real instructions stay within
    budget: every PSUM->SBUF copy runs on DVE (so PSUM-ring releases collapse
    into the one DVE semaphore PE already waits on), and PE "touches" every
    DMA-loaded tensor with a 1-element matmul before real use.
  - engine writes at partition offsets must be 32-aligned, so per-chunk
    column-sum results are collected on partition 0 of a [1, B] row and
    reshaped to [NN, 512] by an SBUF->SBUF DMA.
"""

import os
from contextlib import ExitStack

import numpy as np
import ml_dtypes

import concourse.bass as bass
import concourse.bacc as bacc
import concourse.mybir as mybir
import concourse.tile as tile
from concourse.bass_utils import run_bass_kernel_spmd

F32 = mybir.dt.float32
F32R = mybir.dt.float32r
BF16 = mybir.dt.bfloat16
FP8 = mybir.dt.float8e4
AF = mybir.ActivationFunctionType
ALU = mybir.AluOpType
AX = mybir.AxisListType

B, D, C = 4096, 512, 512
NCORES = 8
RPC = B // NCORES            # rows per core = 512
NCH = 512                    # column chunk size (PSUM bank = 512 fp32)
MARGIN = 0.2
BIG = 4.0

# main-matmul dtype: "f32" (exact, 4 cyc/row) or "f32r" (full rate, ~fp32 acc)
MAIN_DTYPE = os.environ.get("TRIPLET_MAIN_DTYPE", "f32r")


def build_program(Bf=B, Df=D, Cf=C, rpc=RPC, main_dtype=MAIN_DTYPE):
    assert Df % 128 == 0 and Cf % 128 == 0 and Bf % NCH == 0
    assert rpc % 128 == 0 and rpc == NCH, "own-block layout assumes rpc == chunk"
    KD, KC = Df // 128, Cf // 128
    NM = rpc // 128          # 128-row tiles per core
    NN = Bf // NCH           # column chunks
    assert NN % 2 == 0 or NN == 1
    H = Bf // 2 if NN > 1 else Bf

    mm_dt = F32R if main_dtype == "f32r" else F32
    nc = bacc.Bacc("TRN2", target_bir_lowering=False, debug=False)
    ET_d = nc.declare_dram_parameter("ET", [Df, Bf], mm_dt, isOutput=False)
    OTn_d = nc.declare_dram_parameter("OTn", [Cf, Bf], FP8, isOutput=False)
    OTp_d = nc.declare_dram_parameter("OTp", [Cf, rpc], FP8, isOutput=False)
    val_d = nc.declare_dram_parameter("valid", [128, NM], F32, isOutput=False)
    out_d = nc.declare_dram_parameter("out", [1, NM], F32, isOutput=True)

    with tile.TileContext(nc) as tc, ExitStack() as ctx:
        const = ctx.enter_context(tc.tile_pool(name="const", bufs=1))
        big = ctx.enter_context(tc.tile_pool(name="big", bufs=KD))
        sqp = ctx.enter_context(tc.tile_pool(name="sq", bufs=10))
        otnp = ctx.enter_context(tc.tile_pool(name="otn", bufs=1))
        smalls = ctx.enter_context(tc.tile_pool(name="small", bufs=1))
        psA = ctx.enter_context(tc.tile_pool(name="psA", bufs=2, space="PSUM"))
        psB = ctx.enter_context(tc.tile_pool(name="psB", bufs=2, space="PSUM"))
        psM = ctx.enter_context(tc.tile_pool(name="psM", bufs=4, space="PSUM"))

        def pe_touch(ap, ap2=None):
            """1-element matmul so PE observes a tensor producer's semaphore."""
            t = psA.tile([1, NCH], F32, tag="colsum", name="touch")
            nc.tensor.matmul(
                t[0:1, 0:1], lhsT=ap, rhs=ap2 if ap2 is not None else ap,
                start=True, stop=True,
            )

        # constants
        ones_cb = const.tile([128, 1], BF16, tag="ones_cb")
        nc.vector.memset(ones_cb[:], 1.0)
        ones_r = const.tile([1, 128], F32, tag="ones_r")
        nc.vector.memset(ones_r[:], 1.0)
        ones_cf = const.tile([128, 1], F32, tag="ones_cf")
        nc.vector.memset(ones_cf[:], 1.0)
        relu_bias = const.tile([128, 1], F32, tag="relu_bias")
        nc.vector.memset(relu_bias[:], MARGIN - BIG)
        val_t = const.tile([128, NM], F32, tag="val")
        nc.sync.dma_start(val_t[:], val_d[:, :])

        # ---- loads: ET h0, OTp, OTn h0, ET h1, OTn h1 ------------------------
        # (columns are host-permuted per core so chunk 0 is the core's own
        # anchor block: no core-dependent slicing anywhere on device)
        et_tiles = [
            big.tile([128, Bf], mm_dt, tag="big", name=f"et{k}") for k in range(KD)
        ]
        otn_tiles = [
            otnp.tile([128, Bf], FP8, tag=f"otn{k}", name=f"otn{k}") for k in range(KC)
        ]
        otp_tiles = [
            smalls.tile([128, rpc], FP8, tag=f"otp{k}", name=f"otp{k}")
            for k in range(KC)
        ]
        for k in range(KD):
            nc.sync.dma_start(et_tiles[k][:, 0:H], ET_d[k * 128 : (k + 1) * 128, 0:H])
        if H < Bf:
            for k in range(KD):
                nc.sync.dma_start(
                    et_tiles[k][:, H:Bf], ET_d[k * 128 : (k + 1) * 128, H:Bf]
                )
        for k in range(KC):
            nc.sync.dma_start(otp_tiles[k][:], OTp_d[k * 128 : (k + 1) * 128, :])
        for k in range(KC):
            nc.sync.dma_start(otn_tiles[k][:, 0:H], OTn_d[k * 128 : (k + 1) * 128, 0:H])
        if H < Bf:
            for k in range(KC):
                nc.sync.dma_start(
                    otn_tiles[k][:, H:Bf], OTn_d[k * 128 : (k + 1) * 128, H:Bf]
                )


        # ---- per half: column ssq -> r -> broadcast -> in-place normalize ----
        # Emission order interleaves the half-1 normalization with the first
        # main-loop column groups so the DVE never serializes all scaling
        # ahead of the PSUM reductions (engines execute their static order).
        halves = [(0, NN)] if NN == 1 else [(0, NN // 2), (NN // 2, NN // 2)]
        row_buf = smalls.tile([1, Bf], F32, tag="rowbuf")
        r_row = smalls.tile([1, Bf], F32, tag="rrow")
        eh_tiles = et_tiles

        def emit_colsums(cl, cw, split_dve):
            for j in range(cl, cl + cw):
                ps = psA.tile([1, NCH], F32, tag="colsum", name="cs")
                for k in range(KD):
                    sq = sqp.tile([128, NCH], BF16, tag="sq", name="sq")
                    src_ap = et_tiles[k][:, bass.ts(j, NCH)]
                    if split_dve and k % 2 == 1:
                        nc.vector.tensor_tensor(sq[:], src_ap, src_ap, ALU.mult)
                    else:
                        nc.scalar.activation(sq[:], src_ap, AF.Square)
                    nc.tensor.matmul(
                        ps[:], lhsT=ones_cb[:], rhs=sq[:],
                        start=(k == 0), stop=(k == KD - 1),
                    )
                nc.scalar.copy(row_buf[0:1, bass.ts(j, NCH)], ps[:])

        def emit_rsqrt(h, cl, cw):
            ssq = smalls.tile([cw, NCH], F32, tag=f"ssq{h}", name=f"ssq{h}")
            nc.gpsimd.dma_start(ssq[:, :], row_buf[0:1, cl * NCH : (cl + cw) * NCH])
            nrm = smalls.tile([cw, NCH], F32, tag=f"nrm{h}", name=f"nrm{h}")
            nc.scalar.sqrt(nrm[:], ssq[:])
            r0 = smalls.tile([cw, NCH], F32, tag=f"r0{h}", name=f"r0{h}")
            nc.vector.reciprocal_approx_fast(r0[:], nrm[:])
            t1 = smalls.tile([cw, NCH], F32, tag=f"nt1{h}", name=f"nt1{h}")
            nc.vector.tensor_tensor(t1[:], r0[:], r0[:], ALU.mult)
            t2 = smalls.tile([cw, NCH], F32, tag=f"nt2{h}", name=f"nt2{h}")
            nc.vector.tensor_tensor(t2[:], t1[:], ssq[:], ALU.mult)
            nc.vector.tensor_scalar(t2[:], t2[:], -0.5, 1.5, ALU.mult, ALU.add)
            r8 = smalls.tile([cw, NCH], F32, tag=f"r8{h}", name=f"r8{h}")
            nc.vector.tensor_tensor(r8[:], r0[:], t2[:], ALU.mult)
            nc.gpsimd.dma_start(r_row[0:1, cl * NCH : (cl + cw) * NCH], r8[:, :])

        def emit_scale(j):
            rb_ps = psB.tile([128, NCH], F32, tag="rb", name="rb")
            nc.tensor.matmul(
                rb_ps[:], lhsT=ones_r[:], rhs=r_row[0:1, bass.ts(j, NCH)],
                start=True, stop=True,
            )
            for k in range(KD):
                nc.vector.tensor_tensor(
                    eh_tiles[k][:, bass.ts(j, NCH)],
                    et_tiles[k][:, bass.ts(j, NCH)], rb_ps[:], ALU.mult,
                )

        # ---- main loop emission, interleaved with half-1 normalization -------
        loss_all = smalls.tile([128, NM], F32, tag="lossall")
        mps = [
            smalls.tile([128, NN], F32, tag=f"mp{m}", name=f"mp{m}")
            for m in range(NM)
        ]
        mxs = [
            smalls.tile([128, NN], F32, tag=f"mx{m}", name=f"mx{m}")
            for m in range(NM)
        ]

        def emit_blocks(n):
            for m in range(NM):
                ps = psM.tile([128, NCH], F32, tag="ps", name="ps")
                for k in range(KD):
                    nc.tensor.matmul(
                        ps[:],
                        lhsT=eh_tiles[k][:, bass.ts(m, 128)],
                        rhs=eh_tiles[k][:, bass.ts(n, NCH)],
                        start=(k == 0), stop=False,
                    )
                for k in range(KC):
                    nc.tensor.matmul(
                        ps[:],
                        lhsT=otp_tiles[k][:, bass.ts(m, 128)],
                        rhs=otn_tiles[k][:, bass.ts(n, NCH)],
                        start=False, stop=(k == KC - 1),
                    )
                nc.vector.tensor_reduce(mps[m][:, n : n + 1], ps[:], AX.X, ALU.min)
                nc.vector.tensor_reduce(mxs[m][:, n : n + 1], ps[:], AX.X, ALU.max)

        (cl0, cw0) = halves[0]
        emit_colsums(cl0, cw0, split_dve=True)
        emit_rsqrt(0, cl0, cw0)
        # pipelined: scale chunk n, then its column group; the half-1 column
        # sums slot in after the first group and its rsqrt chain after the
        # second, pacing each engine's static order with runtime readiness
        rsqrt1_at = min(2, NN - 1) if len(halves) > 1 else None
        for n in range(NN):
            if len(halves) > 1 and n == 1:
                emit_colsums(halves[1][0], halves[1][1], split_dve=True)
            if rsqrt1_at is not None and n == rsqrt1_at:
                emit_rsqrt(1, halves[1][0], halves[1][1])
            emit_scale(n)
            emit_blocks(n)

        for m in range(NM):
            mpm = smalls.tile([128, 1], F32, tag=f"mpm{m}")
            nc.vector.tensor_reduce(mpm[:], mps[m][:, :], AX.X, ALU.min)
            mxm = smalls.tile([128, 1], F32, tag=f"mxm{m}")
            nc.vector.tensor_reduce(mxm[:], mxs[m][:, :], AX.X, ALU.max)
            dlt = smalls.tile([128, 1], F32, tag=f"dlt{m}")
            nc.vector.tensor_tensor(dlt[:], mxm[:], mpm[:], ALU.subtract)
            rl = smalls.tile([128, 1], F32, tag=f"rl{m}")
            nc.scalar.activation(rl[:], dlt[:], AF.Relu, bias=relu_bias[:])
            nc.vector.tensor_tensor(
                loss_all[:, m : m + 1], rl[:], val_t[:, m : m + 1], ALU.mult
            )

        # ---- partition-sum of per-anchor losses ------------------------------
        out_ps = psA.tile([1, NM], F32, tag="colsum", name="out_ps")
        nc.tensor.matmul(
            out_ps[:], lhsT=ones_cf[:], rhs=loss_all[:, :], start=True, stop=True
        )
        out_sb = smalls.tile([1, NM], F32, tag="outsb")
        nc.vector.tensor_copy(out_sb[:], out_ps[:])
        nc.sync.dma_start(out_d[:, :], out_sb[:])

    nc.compile()
    return nc


def host_prepare(embeddings, labels, Bf=B, Df=D, Cf=C, rpc=RPC):
    """Host-side layout prep + per-core input maps (no embedding math)."""
    embeddings = np.asarray(embeddings, dtype=np.float32)
    labels = np.asarray(labels).astype(np.int64)
    ncores = Bf // rpc
    NM = rpc // 128
    NN = Bf // NCH

    ET = np.ascontiguousarray(embeddings.T)                       # [D, B]
    oh = (np.arange(Cf, dtype=np.int64)[:, None] == labels[None, :])  # [C, B]
    OTn = np.ascontiguousarray((-2.0 * oh).astype(ml_dtypes.float8_e4m3))
    OTp_full = (2.0 * oh).astype(ml_dtypes.float8_e4m3)

    cnt = np.bincount(labels, minlength=Cf)[labels]               # class size per anchor
    valid = ((cnt >= 2) & (cnt <= Bf - 1)).astype(np.float32)     # [B]

    in_maps = []
    for c in range(ncores):
        rows = slice(c * rpc, (c + 1) * rpc)
        # per-core column permutation: own chunk first (chunk 0 on device)
        order = [c] + [j for j in range(NN) if j != c]
        colperm = np.concatenate([np.arange(j * NCH, (j + 1) * NCH) for j in order])
        in_maps.append(
            {
                "ET": np.ascontiguousarray(ET[:, colperm]),
                "OTn": np.ascontiguousarray(OTn[:, colperm]),
                "OTp": np.ascontiguousarray(OTp_full[:, rows]),
                "valid": np.ascontiguousarray(valid[rows].reshape(NM, 128).T),
            }
        )
    return in_maps, valid


_prog_cache = {}


def _get_program():
    key = (B, D, C, RPC, MAIN_DTYPE)
    if key not in _prog_cache:
        _prog_cache[key] = build_program()
    return _prog_cache[key]


LAST_RESULT = None


def kernel(embeddings, labels):
    global LAST_RESULT
    in_maps, valid = host_prepare(embeddings, labels)
    nc = _get_program()
    trace = bool(int(os.environ.get("TRIPLET_TRACE", "0")))
    res = run_bass_kernel_spmd(nc, in_maps, list(range(NCORES)), trace=trace)
    LAST_RESULT = res
    loss_sum = float(sum(r["out"].astype(np.float64).sum() for r in res.results))
    n_valid = max(int(valid.sum()), 1)
    return np.array(loss_sum / n_valid, dtype=np.float32)



# revision 3
# speedup vs baseline: 2.1465x; 2.1465x over previous
"""BatchHardTripletLoss on 8 trn2 NeuronCores (Bass/Tile, SPMD data-parallel).

v2 design (fp8 DoubleRow Gram + label-sorted sparse masking):

Host: rows are sorted by label, L2-normalized, scaled by S=16 and quantized to
fp8e4m3.  Each core owns 512 consecutive sorted anchor rows and computes the
[512, 4096] block of the (scaled) Gram matrix  S^2 * (e_i . e_j)  with fp8
DoubleRow matmuls (K=256 per instruction, 2 instructions per 128x512 block).

Label masking: because rows are sorted, all same-label (positive) pairs of a
core's rows live in at most `nmask` column chunks (the core's own chunk plus
its sorted neighbors).  The host permutes column chunks per core so those
chunks sit at positions 0..nmask-1; a single K<=128 one-hot matmul per
(row-tile, mask-chunk) adds  -4*S^2 * [l_i == l_j]  there (classes per
128-row tile <= 128, so one k-tile always suffices).  Then per anchor row:

    min(row) = S^2 * (hardest-positive-sim - 4)   (shift separates pos below neg)
    max(row) = S^2 * (hardest-negative-sim)
    per-anchor loss = relu(max - min + (margin-4)*S^2) * valid   [S^2-scaled]

PSUM blocks are drained to SBUF bf16 by the ACT engine (most chunks) and DVE
(some chunks, to balance), then DVE reduces min+max over each row-tile's
[128, 4096] bf16 strip in its 4x fast mode.  Host divides the summed result
by S^2 * n_valid.  Validity (>=1 other positive and >=1 negative) depends only
on labels and is computed host-side, shipped as a 0/1 mask.
"""

import os
from contextlib import ExitStack

import numpy as np
import ml_dtypes

import concourse.bass as bass
import concourse.bacc as bacc
import concourse.mybir as mybir
import concourse.tile as tile
from concourse.bass_utils import run_bass_kernel_spmd

F32 = mybir.dt.float32
BF16 = mybir.dt.bfloat16
FP8 = mybir.dt.float8e4
AF = mybir.ActivationFunctionType
ALU = mybir.AluOpType
AX = mybir.AxisListType
DR = mybir.MatmulPerfMode.DoubleRow
FP8NP = ml_dtypes.float8_e4m3

B, D, C = 4096, 512, 512
NCORES = 8
RPC = B // NCORES            # rows per core = 512
NCH = 512                    # column chunk size (PSUM bank = 512 fp32)
NM = RPC // 128              # 128-row tiles per core = 4
NN = B // NCH                # column chunks = 8
KD = D // 128                # contraction k-subtiles = 4
S = 16.0                     # fp8 quantization scale
S2 = S * S
MARGIN = 0.2
BIG = 4.0


def build_program(nmask=3):
    nc = bacc.Bacc("TRN2", target_bir_lowering=False, debug=False)
    ET_d = nc.declare_dram_parameter("ET", [128, NN * KD * NCH], FP8, isOutput=False)
    MP_d = nc.declare_dram_parameter("MP", [128, NM * 128], FP8, isOutput=False)
    MN_d = nc.declare_dram_parameter("MN", [128, NM * nmask * NCH], FP8, isOutput=False)
    val_d = nc.declare_dram_parameter("valid", [128, NM], F32, isOutput=False)
    out_d = nc.declare_dram_parameter("out", [1, NM], F32, isOutput=True)

    with tile.TileContext(nc) as tc, ExitStack() as ctx:
        const = ctx.enter_context(tc.tile_pool(name="const", bufs=1))
        bigp = ctx.enter_context(tc.tile_pool(name="bigp", bufs=1))
        gp = ctx.enter_context(tc.tile_pool(name="gp", bufs=1))
        sm = ctx.enter_context(tc.tile_pool(name="small", bufs=1))
        psM = ctx.enter_context(tc.tile_pool(name="psM", bufs=8, space="PSUM"))

        # constants
        relu_bias = const.tile([128, 1], F32, tag="rbias")
        nc.vector.memset(relu_bias[:], (MARGIN - BIG) * S2)
        ones_cf = const.tile([128, 1], F32, tag="ones")
        nc.vector.memset(ones_cf[:], 1.0)
        val_t = const.tile([128, NM], F32, tag="val")
        nc.sync.dma_start(val_t[:], val_d[:, :])

        # input tiles
        et = bigp.tile([128, NN, KD, NCH], FP8, tag="et")
        mp = sm.tile([128, NM, 128], FP8, tag="mp")
        mnt = sm.tile([128, NM, nmask, NCH], FP8, tag="mn")

        # DMA order: first-needed first.  Chunk n of ET is contiguous
        # [128, KD*NCH] in DRAM (chunk-major host layout).
        CW = KD * NCH
        nc.sync.dma_start(mp[:, :, :], MP_d[:, :])
        nc.sync.dma_start(mnt[:, 0, :, :], MN_d[:, 0:nmask * NCH])
        nc.sync.dma_start(et[:, 0, :, :], ET_d[:, 0:CW])
        nc.sync.dma_start(et[:, 1, :, :], ET_d[:, CW:2 * CW])
        for m in range(1, NM):
            nc.sync.dma_start(
                mnt[:, m, :, :], MN_d[:, m * nmask * NCH:(m + 1) * nmask * NCH]
            )
        for n in range(2, NN):
            nc.sync.dma_start(et[:, n, :, :], ET_d[:, n * CW:(n + 1) * CW])

        # Gram strips (bf16) and per-tile min/max
        g = [
            gp.tile([128, B], BF16, tag=f"g{m}", name=f"g{m}") for m in range(NM)
        ]
        rmn = sm.tile([128, NM], F32, tag="rmn")
        rmx = sm.tile([128, NM], F32, tag="rmx")
        loss_all = sm.tile([128, NM], F32, tag="loss")

        def dve_drains(m):
            # chunks drained by DVE instead of ACT (load balance)
            return (3, 6) if m % 2 == 0 else (4,)

        for m in range(NM):
            for n in range(NN):
                ps = psM.tile([128, NCH], F32, tag="ps", name=f"ps{m}_{n}")
                for kp in range(KD // 2):
                    nc.tensor.matmul(
                        ps[:],
                        lhsT=et[:, 0, 2 * kp:2 * kp + 2, m * 128:(m + 1) * 128],
                        rhs=et[:, n, 2 * kp:2 * kp + 2, :],
                        start=(kp == 0),
                        stop=(kp == KD // 2 - 1 and n >= nmask),
                        perf_mode=DR,
                    )
                if n < nmask:
                    nc.tensor.matmul(
                        ps[:],
                        lhsT=mp[:, m, :],
                        rhs=mnt[:, m, n, :],
                        start=False,
                        stop=True,
                    )
                dst = g[m][:, n * NCH:(n + 1) * NCH]
                if n in dve_drains(m):
                    nc.vector.tensor_copy(dst, ps[:])
                else:
                    nc.scalar.copy(dst, ps[:])
            nc.vector.tensor_reduce(rmn[:, m:m + 1], g[m][:, :], AX.X, ALU.min)
            nc.vector.tensor_reduce(rmx[:, m:m + 1], g[m][:, :], AX.X, ALU.max)

        # tail: per-anchor loss = relu(max - min + (margin-4)*S2) * valid
        for m in range(NM):
            dlt = sm.tile([128, 1], F32, tag=f"dlt{m}")
            nc.vector.tensor_tensor(
                dlt[:], rmx[:, m:m + 1], rmn[:, m:m + 1], ALU.subtract
            )
            rl = sm.tile([128, 1], F32, tag=f"rl{m}")
            nc.scalar.activation(rl[:], dlt[:], AF.Relu, bias=relu_bias[:])
            nc.vector.tensor_tensor(
                loss_all[:, m:m + 1], rl[:], val_t[:, m:m + 1], ALU.mult
            )

        out_ps = psM.tile([128, NCH], F32, tag="ps", name="out_ps")
        nc.tensor.matmul(
            out_ps[0:1, 0:NM], lhsT=ones_cf[:], rhs=loss_all[:, :],
            start=True, stop=True,
        )
        out_sb = sm.tile([1, NM], F32, tag="outsb")
        nc.vector.tensor_copy(out_sb[:], out_ps[0:1, 0:NM])
        nc.sync.dma_start(out_d[:, :], out_sb[:])

    nc.compile()
    return nc


def host_prepare(embeddings, labels):
    """Sort by label, normalize+quantize, build per-core layouts and masks."""
    E = np.asarray(embeddings, dtype=np.float32)
    lab = np.asarray(labels).astype(np.int64)
    order0 = np.argsort(lab, kind="stable")
    ls = lab[order0]
    Es = E[order0]
    nrm = np.maximum(np.linalg.norm(Es, axis=1, keepdims=True), 1e-12)
    En = Es / nrm
    Q8 = (S * En).astype(FP8NP)                  # [B, D]
    QT = np.ascontiguousarray(Q8.T)              # [D, B]

    cnt = np.bincount(ls, minlength=int(ls.max()) + 1)[ls]
    valid_s = ((cnt >= 2) & (cnt <= B - 1)).astype(np.float32)
    n_valid = max(int(valid_s.sum()), 1)

    orders, needs = [], []
    for c in range(NCORES):
        rows = ls[c * RPC:(c + 1) * RPC]
        lo = int(np.searchsorted(ls, rows[0], side="left"))
        hi = int(np.searchsorted(ls, rows[-1], side="right"))
        need = list(range(lo // NCH, (hi - 1) // NCH + 1))
        order = (
            [c]
            + [n for n in need if n != c]
            + [n for n in range(NN) if n != c and n not in need]
        )
        assert order[:len(need)] == [c] + [n for n in need if n != c] or c in need
        orders.append(order)
        needs.append(need)
    nmask = max(3, max(len(n) for n in needs))

    in_maps = []
    for c in range(NCORES):
        order = orders[c]
        rows = ls[c * RPC:(c + 1) * RPC]
        ETc = np.empty((128, NN, KD, NCH), dtype=FP8NP)
        for pos, n in enumerate(order):
            blk = QT[:, n * NCH:(n + 1) * NCH]   # [D, NCH]
            ETc[:, pos] = blk.reshape(KD, 128, NCH).transpose(1, 0, 2)
        MP = np.zeros((128, NM, 128), dtype=FP8NP)
        MN = np.zeros((128, NM, nmask, NCH), dtype=FP8NP)
        for m in range(NM):
            rl = rows[m * 128:(m + 1) * 128]
            cm, w_inv = np.unique(rl, return_inverse=True)
            MP[w_inv, m, np.arange(128)] = FP8NP(32.0)
            for pos in range(min(nmask, NN)):
                n = order[pos]
                colsn = ls[n * NCH:(n + 1) * NCH]
                match = cm[:, None] == colsn[None, :]
                MN[:len(cm), m, pos, :][match] = FP8NP(-32.0)
        vmat = np.ascontiguousarray(
            valid_s[c * RPC:(c + 1) * RPC].reshape(NM, 128).T
        )
        in_maps.append(
            {
                "ET": np.ascontiguousarray(ETc.reshape(128, NN * KD * NCH)),
                "MP": np.ascontiguousarray(MP.reshape(128, NM * 128)),
                "MN": np.ascontiguousarray(MN.reshape(128, NM * nmask * NCH)),
                "valid": vmat,
            }
        )
    return in_maps, n_valid, nmask


_prog_cache = {}


def _get_program(nmask):
    key = (B, D, RPC, nmask)
    if key not in _prog_cache:
        _prog_cache[key] = build_program(nmask)
    return _prog_cache[key]


LAST_RESULT = None


def kernel(embeddings, labels):
    global LAST_RESULT
    in_maps, n_valid, nmask = host_prepare(embeddings, labels)
    nc = _get_program(nmask)
    trace = bool(int(os.environ.get("TRIPLET_TRACE", "0")))
    res = run_bass_kernel_spmd(nc, in_maps, list(range(NCORES)), trace=trace)
    LAST_RESULT = res
    loss_sum = float(sum(r["out"].astype(np.float64).sum() for r in res.results))
    return np.array(loss_sum / (S2 * n_valid), dtype=np.float32)


# revision 7
# speedup vs baseline: 3.1414x; 1.4635x over previous
"""BatchHardTripletLoss on 8 trn2 NeuronCores (Bass/Tile, SPMD data-parallel).

v4 design (fp8 DoubleRow Gram + label-sorted sparse masking + wide ACT
drains + DVE bf16 max-tree):

Host: rows are sorted by label, L2-normalized, scaled by S=16 and quantized to
fp8e4m3.  Each core owns 512 consecutive sorted anchor rows and computes the
[512, 4096] block of the (scaled) Gram matrix  S^2 * (e_i . e_j)  with fp8
DoubleRow matmuls (K=256 per instruction, 2 per 128x512 sub-block).

Label masking: because rows are sorted, all same-label (positive) pairs of a
core's rows live in the core's own column chunk plus its sorted neighbors.
The host permutes column chunks per core so those sit at positions 0..2; a
single K<=128 one-hot matmul per (row-tile, mask-chunk) adds
-4*S^2*[l_i == l_j] there.  Mask matmuls are emitted only for (m, n) blocks
where some core has a shared class (union across cores; zero one-hots are
harmless for the others).

Pipeline per 128-row tile m: 4 two-bank PSUM tiles [128, 1024] are filled by
2x2 DR matmul groups, drained by single wide ACT copies into a bf16 strip
g[m] [128, 4096].  DVE then runs a tensor_tensor max tree (bf16 2x mode):
t1a=max(g0,g1), t1b=max(g2,g3), t2=max(t1a,t1b), ttr(t2 halves)->rmx.
The min scan (hardest positive) only covers the <=3 chunks that can contain
positives (union across cores): a chained ttr/tt over those strips -> rmn.
Unshifted entries are >= -S^2, far above any shifted positive, so extra
chunks in the min scan are harmless.

Tail (merged across m): loss = relu(rmx - rmn + (margin-4)*S^2) * valid,
summed by a ones-matmul; host divides by S^2 * n_valid.  Validity depends
only on labels and is computed host-side.
"""

import os
from contextlib import ExitStack

import numpy as np
import ml_dtypes

import concourse.bass as bass
import concourse.bacc as bacc
import concourse.mybir as mybir
import concourse.tile as tile
from concourse.bass_utils import run_bass_kernel_spmd

F32 = mybir.dt.float32
BF16 = mybir.dt.bfloat16
FP8 = mybir.dt.float8e4
AF = mybir.ActivationFunctionType
ALU = mybir.AluOpType
AX = mybir.AxisListType
DR = mybir.MatmulPerfMode.DoubleRow
FP8NP = ml_dtypes.float8_e4m3

B, D, C = 4096, 512, 512
NCORES = 8
RPC = B // NCORES            # rows per core = 512
NCH = 512                    # column chunk size (PSUM bank = 512 fp32)
NM = RPC // 128              # 128-row tiles per core = 4
NN = B // NCH                # column chunks = 8
KD = D // 128                # contraction k-subtiles = 4
S = 16.0                     # fp8 quantization scale
S2 = S * S
MARGIN = 0.2
BIG = 4.0


def build_program(nmask, mask_blocks, min_chunks):
    """mask_blocks: frozenset of (m, n) needing a one-hot mask matmul.
    min_chunks: tuple over m of tuple of chunk positions the min must scan."""
    nc = bacc.Bacc("TRN2", target_bir_lowering=False, debug=False)
    ET_d = nc.declare_dram_parameter("ET", [128, NN * KD * NCH], FP8, isOutput=False)
    MP_d = nc.declare_dram_parameter("MP", [128, NM * 128], FP8, isOutput=False)
    MN_d = nc.declare_dram_parameter("MN", [128, NM * nmask * NCH], FP8, isOutput=False)
    val_d = nc.declare_dram_parameter("valid", [128, NM], F32, isOutput=False)
    out_d = nc.declare_dram_parameter("out", [1, NM], F32, isOutput=True)

    with tile.TileContext(nc) as tc, ExitStack() as ctx:
        const = ctx.enter_context(tc.tile_pool(name="const", bufs=1))
        bigp = ctx.enter_context(tc.tile_pool(name="bigp", bufs=1))
        gp = ctx.enter_context(tc.tile_pool(name="gp", bufs=1))
        scr = ctx.enter_context(tc.tile_pool(name="scr", bufs=2))
        sm = ctx.enter_context(tc.tile_pool(name="small", bufs=1))
        psM = ctx.enter_context(tc.tile_pool(name="psM", bufs=4, space="PSUM"))

        # constants
        relu_bias = const.tile([128, 1], F32, tag="rbias")
        nc.vector.memset(relu_bias[:], (MARGIN - BIG) * S2)
        ones_cf = const.tile([128, 1], F32, tag="ones")
        nc.vector.memset(ones_cf[:], 1.0)
        val_t = const.tile([128, NM], F32, tag="val")
        nc.sync.dma_start(val_t[:], val_d[:, :])

        # input tiles
        et = bigp.tile([128, NN, KD, NCH], FP8, tag="et")
        mp = sm.tile([128, NM, 128], FP8, tag="mp")
        mnt = sm.tile([128, NM, nmask, NCH], FP8, tag="mn")

        # DMA order: first-needed first.  Chunk n of ET is contiguous
        # [128, KD*NCH] in DRAM (chunk-major host layout).
        CW = KD * NCH
        nc.sync.dma_start(mp[:, :, :], MP_d[:, :])
        for (m, n) in sorted(mask_blocks):
            if m == 0:
                nc.sync.dma_start(
                    mnt[:, m, n, :],
                    MN_d[:, (m * nmask + n) * NCH:(m * nmask + n + 1) * NCH],
                )
        nc.sync.dma_start(et[:, 0, :, :], ET_d[:, 0:CW])
        nc.sync.dma_start(et[:, 1, :, :], ET_d[:, CW:2 * CW])
        for (m, n) in sorted(mask_blocks):
            if m > 0:
                nc.sync.dma_start(
                    mnt[:, m, n, :],
                    MN_d[:, (m * nmask + n) * NCH:(m * nmask + n + 1) * NCH],
                )
        for n in range(2, NN):
            nc.sync.dma_start(et[:, n, :, :], ET_d[:, n * CW:(n + 1) * CW])

        g = [
            gp.tile([128, B], BF16, tag=f"g{m}", name=f"g{m}") for m in range(NM)
        ]
        rmn = sm.tile([128, NM], F32, tag="rmn")
        rmx = sm.tile([128, NM], F32, tag="rmx")

        for m in range(NM):
            for j in range(NN // 2):
                ps = psM.tile([128, 2 * NCH], F32, tag="ps", name=f"ps{m}_{j}")
                for h in range(2):
                    n = 2 * j + h
                    has_mask = (m, n) in mask_blocks
                    dst = ps[:, h * NCH:(h + 1) * NCH]
                    for kp in range(KD // 2):
                        nc.tensor.matmul(
                            dst,
                            lhsT=et[:, 0, 2 * kp:2 * kp + 2, m * 128:(m + 1) * 128],
                            rhs=et[:, n, 2 * kp:2 * kp + 2, :],
                            start=(kp == 0),
                            stop=(kp == KD // 2 - 1 and not has_mask),
                            perf_mode=DR,
                        )
                    if has_mask:
                        nc.tensor.matmul(
                            dst,
                            lhsT=mp[:, m, :],
                            rhs=mnt[:, m, n, :],
                            start=False,
                            stop=True,
                        )
                nc.scalar.copy(g[m][:, j * 2 * NCH:(j + 1) * 2 * NCH], ps[:])
            # max tree on bf16 strips (tensor_tensor 2x mode)
            t1a = scr.tile([128, 2 * NCH], BF16, tag="t1a", name=f"t1a{m}")
            nc.vector.tensor_tensor(
                t1a[:], g[m][:, 0:2 * NCH], g[m][:, 2 * NCH:4 * NCH], ALU.max
            )
            t1b = scr.tile([128, 2 * NCH], BF16, tag="t1b", name=f"t1b{m}")
            nc.vector.tensor_tensor(
                t1b[:], g[m][:, 4 * NCH:6 * NCH], g[m][:, 6 * NCH:8 * NCH], ALU.max
            )
            t2 = scr.tile([128, 2 * NCH], BF16, tag="t2", name=f"t2{m}")
            nc.vector.tensor_tensor(t2[:], t1a[:], t1b[:], ALU.max)
            t3 = scr.tile([128, NCH], BF16, tag="t3", name=f"t3{m}")
            nc.vector.tensor_tensor(
                t3[:], t2[:, 0:NCH], t2[:, NCH:2 * NCH], ALU.max
            )
            nc.vector.tensor_reduce(rmx[:, m:m + 1], t3[:], AX.X, ALU.max)
            # min over the chunks that can contain positives (contiguous span)
            mc = min_chunks[m]
            lo, hi = min(mc), max(mc)
            span = hi - lo + 1
            if span == 1:
                nc.vector.tensor_reduce(
                    rmn[:, m:m + 1], g[m][:, lo * NCH:(lo + 1) * NCH],
                    AX.X, ALU.min,
                )
            else:
                x0 = scr.tile([128, NCH], BF16, tag="x0", name=f"x0{m}")
                nc.vector.tensor_tensor(
                    x0[:],
                    g[m][:, lo * NCH:(lo + 1) * NCH],
                    g[m][:, (lo + 1) * NCH:(lo + 2) * NCH],
                    ALU.min,
                )
                for e in range(2, span):
                    nc.vector.tensor_tensor(
                        x0[:], x0[:],
                        g[m][:, (lo + e) * NCH:(lo + e + 1) * NCH],
                        ALU.min,
                    )
                nc.vector.tensor_reduce(
                    rmn[:, m:m + 1], x0[:], AX.X, ALU.min
                )

        # merged tail: loss = relu(rmx - rmn + (margin-4)*S2) * valid
        dlt = sm.tile([128, NM], F32, tag="dlt")
        nc.vector.tensor_tensor(dlt[:], rmx[:, :], rmn[:, :], ALU.subtract)
        rl = sm.tile([128, NM], F32, tag="rl")
        nc.scalar.activation(rl[:], dlt[:], AF.Relu, bias=relu_bias[:])
        loss_all = sm.tile([128, NM], F32, tag="loss")
        nc.vector.tensor_tensor(loss_all[:], rl[:], val_t[:, :], ALU.mult)

        out_ps = psM.tile([128, 2 * NCH], F32, tag="ps", name="out_ps")
        nc.tensor.matmul(
            out_ps[0:1, 0:NM], lhsT=ones_cf[:], rhs=loss_all[:, :],
            start=True, stop=True,
        )
        out_sb = sm.tile([1, NM], F32, tag="outsb")
        nc.vector.tensor_copy(out_sb[:], out_ps[0:1, 0:NM])
        nc.sync.dma_start(out_d[:, :], out_sb[:])

    nc.compile()
    return nc


def host_prepare(embeddings, labels):
    """Sort by label, normalize+quantize, build per-core layouts and masks."""
    E = np.asarray(embeddings, dtype=np.float32)
    lab = np.asarray(labels).astype(np.int64)
    order0 = np.argsort(lab, kind="stable")
    ls = lab[order0]
    Es = E[order0]
    nrm = np.maximum(np.linalg.norm(Es, axis=1, keepdims=True), 1e-12)
    En = Es / nrm
    Q8 = (S * En).astype(FP8NP)                  # [B, D]
    QT = np.ascontiguousarray(Q8.T)              # [D, B]

    cnt = np.bincount(ls, minlength=int(ls.max()) + 1)[ls]
    valid_s = ((cnt >= 2) & (cnt <= B - 1)).astype(np.float32)
    n_valid = max(int(valid_s.sum()), 1)

    orders, needs = [], []
    for c in range(NCORES):
        rows = ls[c * RPC:(c + 1) * RPC]
        lo = int(np.searchsorted(ls, rows[0], side="left"))
        hi = int(np.searchsorted(ls, rows[-1], side="right"))
        need = list(range(lo // NCH, (hi - 1) // NCH + 1))
        order = (
            [c]
            + ([c - 1] if c > 0 else [])
            + ([c + 1] if c < NN - 1 else [])
        )
        order += [n for n in need if n not in order]
        order += [n for n in range(NN) if n not in order]
        orders.append(order)
        needs.append(need)
    nmask = max(3, max(len(n) for n in needs))

    # structural info shared by all cores (program is SPMD-shared):
    # which (m, n) blocks need a mask matmul, and which chunk positions the
    # min scan must cover per m -- union across cores.
    mask_blocks = set()
    min_chunks = [set() for _ in range(NM)]
    in_maps = []
    for c in range(NCORES):
        order = orders[c]
        pos_of = {n: i for i, n in enumerate(order)}
        rows = ls[c * RPC:(c + 1) * RPC]
        ETc = np.empty((128, NN, KD, NCH), dtype=FP8NP)
        for pos, n in enumerate(order):
            blk = QT[:, n * NCH:(n + 1) * NCH]   # [D, NCH]
            ETc[:, pos] = blk.reshape(KD, 128, NCH).transpose(1, 0, 2)
        MP = np.zeros((128, NM, 128), dtype=FP8NP)
        MN = np.zeros((128, NM, nmask, NCH), dtype=FP8NP)
        for m in range(NM):
            rl = rows[m * 128:(m + 1) * 128]
            cm, w_inv = np.unique(rl, return_inverse=True)
            MP[w_inv, m, np.arange(128)] = FP8NP(32.0)
            # columns of this tile's classes (sorted => contiguous range)
            lo = int(np.searchsorted(ls, rl[0], side="left"))
            hi = int(np.searchsorted(ls, rl[-1], side="right"))
            for n in range(lo // NCH, (hi - 1) // NCH + 1):
                pos = pos_of[n]
                assert pos < nmask, (c, m, n, pos, order)
                colsn = ls[n * NCH:(n + 1) * NCH]
                match = cm[:, None] == colsn[None, :]
                MN[:len(cm), m, pos, :][match] = FP8NP(-32.0)
                mask_blocks.add((m, pos))
                min_chunks[m].add(pos)
        vmat = np.ascontiguousarray(
            valid_s[c * RPC:(c + 1) * RPC].reshape(NM, 128).T
        )
        in_maps.append(
            {
                "ET": np.ascontiguousarray(ETc.reshape(128, NN * KD * NCH)),
                "MP": np.ascontiguousarray(MP.reshape(128, NM * 128)),
                "MN": np.ascontiguousarray(MN.reshape(128, NM * nmask * NCH)),
                "valid": vmat,
            }
        )
    struct = (
        nmask,
        frozenset(mask_blocks),
        tuple(tuple(sorted(min_chunks[m])) for m in range(NM)),
    )
    return in_maps, n_valid, struct


_prog_cache = {}


def _get_program(struct):
    if struct not in _prog_cache:
        _prog_cache[struct] = build_program(*struct)
    return _prog_cache[struct]


LAST_RESULT = None


def kernel(embeddings, labels):
    global LAST_RESULT
    in_maps, n_valid, struct = host_prepare(embeddings, labels)
    nc = _get_program(struct)
    trace = bool(int(os.environ.get("TRIPLET_TRACE", "0")))
    res = run_bass_kernel_spmd(nc, in_maps, list(range(NCORES)), trace=trace)
    LAST_RESULT = res
    loss_sum = float(sum(r["out"].astype(np.float64).sum() for r in res.results))
    return np.array(loss_sum / (S2 * n_valid), dtype=np.float32)
